# revision 6
# baseline (speedup 1.0000x reference)
"""Trainium2 Bass kernel for nn_ExtractorMLP (GNN edge cosine-similarity).

Math:  out[e] = cos_sim(mlp(emb[col[e]]), mlp(emb[row[e]]))
where  mlp(x) = elu(x @ W1.T + b1) @ W2.T + b2   (b1 = b2 = 0 for this problem)

Strategy (per the edge-data-parallel sharding hint):
  * Phase 1 (per node, replicated on every core): compute the normalized MLP
    output table  t[v] = g[v] / max(||g[v]||, eps)  for all N nodes, assembled
    directly in SBUF in the packed layout dma_gather expects (node v lives in
    partition v%128 at free-byte offset (v//128)*256, bf16).
  * Phase 2 (edges, sharded 8 ways): each core gathers t[col], t[row] for its
    edge slice with SBUF-source transpose dma_gather (output lands as
    [feature=128 partitions, edge columns]), multiplies elementwise on DVE and
    contracts over features with a ones-vector matmul on the tensor engine.
    dma_gather indices are int16, so edges are bucketed on the host by
    (col < 32768, row < 32768) and each bucket gathers from the matching
    half of the table with half-local indices.

ELU identity used on device:  elu(x) = max(min(exp(x), 1) - 1, x)
"""

import math

import numpy as np
import ml_dtypes

BF16 = ml_dtypes.bfloat16

H = 128          # feature dim
P = 128          # partitions
CHUNK = 512      # edges per reduce-matmul
GT = 4096        # edges per dma_gather instruction
HALF = 32768     # int16 index limit: table split point
NCORES = 8
ST_W = 512       # phase-1 supertile width (nodes)

_PROG_CACHE: dict = {}
LAST_RESULTS = None  # test harness can inspect exec_time_ns


def _build_program(n_pad, half, nck, trace_label=""):
    """Build the (shared, SPMD) bass program.

    n_pad: padded node count (multiple of 128) = table free-dim elems/partition
    half:  table split element offset (multiple of 128)
    nck:   per-bucket chunk counts (len 4), shared across cores
    """
    import concourse.bacc as bacc
    import concourse.mybir as mybir
    import concourse.tile as tile
    from concourse import library_config
    from contextlib import ExitStack

    f32 = mybir.dt.float32
    bf16 = mybir.dt.bfloat16
    i16 = mybir.dt.int16
    Alu = mybir.AluOpType
    Act = mybir.ActivationFunctionType

    n_chunks = sum(nck)
    n_groups = math.ceil(n_chunks / P)
    stream_cols = n_chunks * (CHUNK // 16)   # wrapped-idx columns

    nc = bacc.Bacc("TRN2", target_bir_lowering=False, debug=False,
                   num_devices=NCORES)

    embT = nc.dram_tensor("embT", [P, n_pad], bf16, kind="ExternalInput")
    w1t_d = nc.dram_tensor("w1t", [H, H], bf16, kind="ExternalInput")
    w2t_d = nc.dram_tensor("w2t", [H, H], bf16, kind="ExternalInput")
    cidx_d = nc.dram_tensor("cidx", [P, stream_cols], i16, kind="ExternalInput")
    ridx_d = nc.dram_tensor("ridx", [P, stream_cols], i16, kind="ExternalInput")
    out_d = nc.dram_tensor("out", [n_groups, P, CHUNK], f32, kind="ExternalOutput")

    with ExitStack() as ctx:
        tc = ctx.enter_context(tile.TileContext(nc))
        const = ctx.enter_context(tc.tile_pool(name="const", bufs=1))
        p1 = ctx.enter_context(tc.tile_pool(name="p1", bufs=3))
        p2 = ctx.enter_context(tc.tile_pool(name="p2", bufs=2))
        pprod = ctx.enter_context(tc.tile_pool(name="pprod", bufs=4))
        ps1 = ctx.enter_context(tc.tile_pool(name="ps1", bufs=2, space="PSUM"))
        ps2 = ctx.enter_context(tc.tile_pool(name="ps2", bufs=2, space="PSUM"))
        pso = ctx.enter_context(tc.tile_pool(name="pso", bufs=2, space="PSUM"))

        nc.gpsimd.load_library(library_config.mlp)

        # --- constants / persistent tiles ---
        table = const.tile([P, n_pad], bf16, tag="table")
        w1t = const.tile([H, H], bf16, tag="w1t")
        w2t = const.tile([H, H], bf16, tag="w2t")
        # sliding one-hot: onehot[:, 127-p : 255-p] has ones in column p only;
        # used as lhsT so chunk p's dot-row lands in PSUM partition p.
        onehot = const.tile([P, 2 * P - 1], bf16, tag="onehot")
        cidx = const.tile([P, stream_cols], i16, tag="cidx")
        ridx = const.tile([P, stream_cols], i16, tag="ridx")
        nc.sync.dma_start(out=w1t[:], in_=w1t_d[:])
        nc.sync.dma_start(out=w2t[:], in_=w2t_d[:])
        nc.sync.dma_start(out=cidx[:], in_=cidx_d[:])
        nc.sync.dma_start(out=ridx[:], in_=ridx_d[:])
        nc.vector.memset(onehot[:], 0.0)
        nc.vector.memset(onehot[:, P - 1:P], 1.0)

        # --- phase 1: build normalized table ---
        n0 = 0
        while n0 < n_pad:
            w = min(ST_W, n_pad - n0)
            nb = w // H
            xt = p1.tile([P, ST_W], bf16, tag="xt", name="xt")[:, :w]
            nc.sync.dma_start(out=xt, in_=embT[:, n0:n0 + w])
            ph1 = ps1.tile([P, ST_W], f32, tag="ph1", name="ph1")[:, :w]
            nc.tensor.matmul(ph1, lhsT=w1t[:], rhs=xt, start=True, stop=True)
            e_t = p1.tile([P, ST_W], bf16, tag="e", name="e")[:, :w]
            nc.scalar.activation(e_t, ph1, Act.Exp)
            e2_t = p1.tile([P, ST_W], bf16, tag="e2", name="e2")[:, :w]
            nc.gpsimd.tensor_scalar_min(e2_t, e_t, 1.0)
            h1_t = p1.tile([P, ST_W], bf16, tag="h1", name="h1")[:, :w]
            nc.vector.scalar_tensor_tensor(
                h1_t, in0=e2_t, scalar=-1.0, in1=ph1,
                op0=Alu.add, op1=Alu.max)
            pg = ps2.tile([P, ST_W], f32, tag="pg", name="pg")[:, :w]
            for b in range(nb):
                nc.tensor.matmul(pg[:, b * H:(b + 1) * H],
                                 lhsT=h1_t[:, b * H:(b + 1) * H],
                                 rhs=w2t[:], start=True, stop=True)
            sq_t = p1.tile([P, ST_W], bf16, tag="sq", name="sq")[:, :w]
            ss_t = p1.tile([P, ST_W // H], f32, tag="ss", name="ss")[:, :nb]
            for b in range(nb):
                nc.scalar.activation(sq_t[:, b * H:(b + 1) * H],
                                     pg[:, b * H:(b + 1) * H],
                                     Act.Square, accum_out=ss_t[:, b:b + 1])
            s_t = p1.tile([P, ST_W // H], f32, tag="s", name="s")[:, :nb]
            nc.scalar.activation(s_t, ss_t, Act.Sqrt)
            m_t = p1.tile([P, ST_W // H], f32, tag="m", name="m")[:, :nb]
            nc.vector.tensor_scalar_max(m_t, s_t, 1e-8)
            r_t = p1.tile([P, ST_W // H], f32, tag="r", name="r")[:, :nb]
            nc.vector.reciprocal(r_t, m_t)
            for b in range(nb):
                nc.vector.tensor_scalar_mul(
                    table[:, n0 + b * H:n0 + (b + 1) * H],
                    pg[:, b * H:(b + 1) * H], r_t[:, b:b + 1])
            n0 += w

        # --- phase 2: gather + per-edge dot products ---
        halves = (table[:, :half], table[:, half:n_pad])
        half_free_bytes = (half * 2, (n_pad - half) * 2)

        chunk_id = 0
        pout = None
        stream0 = 0
        for k in range(4):
            kc, kr = k >> 1, k & 1   # 0 -> half A, 1 -> half B
            bucket_edges = nck[k] * CHUNK
            t0 = 0
            while t0 < bucket_edges:
                tsz = min(GT, bucket_edges - t0)
                w0 = (stream0 + t0) // 16
                w1 = (stream0 + t0 + tsz) // 16
                f1 = p2.tile([P, 1, GT], bf16, tag="f1", name="f1")[:, :, :tsz]
                f2 = p2.tile([P, 1, GT], bf16, tag="f2", name="f2")[:, :, :tsz]
                nc.gpsimd.dma_gather(
                    f1, halves[kc], cidx[:, w0:w1], tsz, tsz, H,
                    transpose=True, sbuf_tokens_per_rank=P,
                    sbuf_free_dim_per_rank=256, single_packet=False)
                nc.gpsimd.dma_gather(
                    f2, halves[kr], ridx[:, w0:w1], tsz, tsz, H,
                    transpose=True, sbuf_tokens_per_rank=P,
                    sbuf_free_dim_per_rank=256, single_packet=False)
                for c in range(tsz // CHUNK):
                    prod = pprod.tile([P, CHUNK], bf16, tag="prod")
                    nc.vector.tensor_tensor(
                        out=prod[:], in0=f1[:, 0, c * CHUNK:(c + 1) * CHUNK],
                        in1=f2[:, 0, c * CHUNK:(c + 1) * CHUNK], op=Alu.mult)
                    g, p = divmod(chunk_id, P)
                    if p == 0:
                        pout = pso.tile([P, CHUNK], f32, tag="pout")
                    last = chunk_id == n_chunks - 1
                    nc.tensor.matmul(pout[:], lhsT=onehot[:, P - 1 - p:2 * P - 1 - p],
                                     rhs=prod[:], start=(p == 0),
                                     stop=(p == P - 1 or last))
                    chunk_id += 1
                    if p == P - 1 or last:
                        rows = p + 1
                        ost = p2.tile([P, CHUNK], f32, tag="ost", name="ost")[:rows]
                        nc.vector.tensor_copy(out=ost, in_=pout[:rows])
                        nc.sync.dma_start(out=out_d[g, :rows], in_=ost)
                t0 += tsz
            stream0 += bucket_edges

    nc.compile()
    return nc


def _wrap_idx(idx):
    """[S*16] int16 -> [128, S] wrapped layout (16 partitions, replicated 8x)."""
    w = idx.reshape(-1, 16).T.astype(np.int16)
    return np.tile(w, (8, 1))


def _ensure_ntff_hook():
    """Provide antenv.axon_hooks if the image lacks it (trace support only)."""
    import sys
    import types
    try:
        import antenv.axon_hooks  # noqa: F401
        return
    except ImportError:
        pass
    try:
        import antenv
        from trn_agent_boot.trn_boot import _ntff_profile_via_ctypes
        mod = types.ModuleType("antenv.axon_hooks")
        mod._hook = _ntff_profile_via_ctypes("/opt/axon/libaxon_pjrt.so")
        mod.get_axon_ntff_profile_hook = lambda: mod._hook
        mod.set_axon_ntff_profile_hook = lambda h: setattr(mod, "_hook", h)
        sys.modules["antenv.axon_hooks"] = mod
        antenv.axon_hooks = mod
    except Exception:
        pass


def kernel(emb, edge_index, W1, b1, W2, b2):
    global LAST_RESULTS
    from concourse.bass_utils import run_bass_kernel_spmd
    _ensure_ntff_hook()

    emb = np.asarray(emb, dtype=np.float32)
    W1 = np.asarray(W1, dtype=np.float32)
    W2 = np.asarray(W2, dtype=np.float32)
    b1 = np.asarray(b1, dtype=np.float32)
    b2 = np.asarray(b2, dtype=np.float32)
    assert np.abs(b1).max() == 0 and np.abs(b2).max() == 0, \
        "nonzero biases not implemented"
    col = np.asarray(edge_index[0]).astype(np.int64)
    row = np.asarray(edge_index[1]).astype(np.int64)

    n, h = emb.shape
    assert h == H
    E = col.shape[0]
    assert E % NCORES == 0
    ec = E // NCORES
    n_pad = ((n + P - 1) // P) * P
    half = min(HALF, n_pad)

    # ---- host prep: per-core bucketed edge streams ----
    cores = []
    for ci in range(NCORES):
        c = col[ci * ec:(ci + 1) * ec]
        r = row[ci * ec:(ci + 1) * ec]
        b = ((c >= half).astype(np.int8) << 1) | (r >= half).astype(np.int8)
        order = np.argsort(b, kind="stable")
        counts = np.bincount(b, minlength=4)
        cores.append((c[order], r[order], order, counts))

    nck = tuple(int(math.ceil(max(cr[3][k] for cr in cores) / CHUNK))
                for k in range(4))
    n_chunks = sum(nck)
    n_groups = math.ceil(n_chunks / P)

    key = (n_pad, half, nck)
    if key not in _PROG_CACHE:
        _PROG_CACHE[key] = _build_program(n_pad, half, nck)
    nc = _PROG_CACHE[key]

    # ---- per-core input maps ----
    embT = np.zeros((P, n_pad), dtype=BF16)
    embT[:, :n] = emb.T.astype(BF16)
    w1t = W1.T.astype(BF16)
    w2t = W2.T.astype(BF16)

    in_maps = []
    for (cs, rs, order, counts) in cores:
        c_stream = np.zeros(n_chunks * CHUNK, dtype=np.int64)
        r_stream = np.zeros(n_chunks * CHUNK, dtype=np.int64)
        off_e = 0   # offset into sorted edge arrays
        off_s = 0   # offset into padded stream
        for k in range(4):
            cnt = counts[k]
            sel = slice(off_e, off_e + cnt)
            c_stream[off_s:off_s + cnt] = cs[sel] - (half if k >> 1 else 0)
            r_stream[off_s:off_s + cnt] = rs[sel] - (half if k & 1 else 0)
            off_e += cnt
            off_s += nck[k] * CHUNK
        in_maps.append({
            "embT": embT, "w1t": w1t, "w2t": w2t,
            "cidx": _wrap_idx(c_stream), "ridx": _wrap_idx(r_stream),
        })

    res = run_bass_kernel_spmd(nc, in_maps, core_ids=list(range(NCORES)))
    LAST_RESULTS = res

    # ---- reassemble ----
    out = np.empty(E, dtype=np.float32)
    for ci, (cs, rs, order, counts) in enumerate(cores):
        stream = res.results[ci]["out"].reshape(-1)   # chunk-major dots
        vals = np.empty(ec, dtype=np.float32)
        off_e = 0
        off_s = 0
        for k in range(4):
            cnt = counts[k]
            vals[off_e:off_e + cnt] = stream[off_s * CHUNK:off_s * CHUNK + cnt]
            off_e += cnt
            off_s += nck[k]
        seg = out[ci * ec:(ci + 1) * ec]
        seg[order] = vals
    return out
